# revision 6
# baseline (speedup 1.0000x reference)
"""Causal self-attention (B=4, S=2048, D=1024, H=16) on 8 TRN2 NeuronCores.

Sharding: batch 4-way x head-group 2-way. Core c handles batch c//2 and
heads (c%2)*8 .. (c%2)*8+8. Each core computes its QKV projection slice,
per-head causal attention, and a partial output projection (W_out rows of
its heads); the host sums the two head-group partials per batch.

All TensorEngine matmuls run in float32r (fp32 storage, fast PE path).
"""

import os
import sys

for _p in ("/opt/trn_rl_repo", "/root/.axon_site/_ro/trn_rl_repo"):
    if os.path.isdir(_p) and _p not in sys.path:
        sys.path.insert(0, _p)

import numpy as np

import concourse.bass as bass  # noqa: E402
import concourse.tile as tile  # noqa: E402
from concourse import bacc, mybir  # noqa: E402
from concourse.bass_utils import run_bass_kernel_spmd  # noqa: E402

F32 = mybir.dt.float32
R32 = mybir.dt.float32r

B = 4
S = 2048
D = 1024
H = 16
HD = 64  # head dim
HLOC = 8  # heads per core
SCALE = HD ** -0.5

C = D // 128  # dm chunks (8)
NQS = S // 512  # q subtiles (4)
NST = S // 128  # s tiles (16)
NT = HLOC * HD // 128  # Q/K partition groups (4)
NFC = NT  # vals f-chunks (4)


def _build_nc(use_bias: bool):
    nc = bacc.Bacc(
        "TRN2",
        target_bir_lowering=False,
        debug=False,
        enable_asserts=True,
        num_devices=8,
    )

    dw = D + 1 if use_bias else D
    xT = nc.dram_tensor("xT", [dw, S], R32, kind="ExternalInput")
    wq = nc.dram_tensor("wq", [dw, 512], R32, kind="ExternalInput")
    wk = nc.dram_tensor("wk", [dw, 512], R32, kind="ExternalInput")
    wv = nc.dram_tensor("wv", [dw, 512], R32, kind="ExternalInput")
    wo = nc.dram_tensor("wo", [512, D], R32, kind="ExternalInput")
    mask = nc.dram_tensor("mask", [128, 4 * 512], R32, kind="ExternalInput")
    y = nc.dram_tensor("y", [S, D], F32, kind="ExternalOutput")

    with tile.TileContext(nc) as tc:
        with tc.tile_pool(name="persist", bufs=1) as persist:
            mask_sb = persist.tile([128, 4, 512], R32)
            nc.sync.dma_start(mask_sb[:], mask.rearrange("p (g q) -> p g q", g=4))
            QT_sb = persist.tile([128, NT, S], R32)
            KT_sb = persist.tile([128, NT, S], R32)
            V_sb = persist.tile([128, NST, HLOC, HD + 1], R32)
            nc.vector.memset(
                V_sb.rearrange("p a b c -> p (a b c)").bitcast(F32), 1.0
            )

            # ---------------- Phase 1: QKV^T projection ----------------
            with (
                tc.tile_pool(name="p1", bufs=1) as p1,
                tc.tile_pool(name="ps1", bufs=1, space="PSUM") as ps1,
            ):
                wq_sb = p1.tile([128, C + 1, 512], R32)
                wk_sb = p1.tile([128, C + 1, 512], R32)
                wv_sb = p1.tile([128, C + 1, 512], R32)
                for wsb, wdr in ((wq_sb, wq), (wk_sb, wk), (wv_sb, wv)):
                    nc.sync.dma_start(
                        wsb[:, 0:C, :],
                        wdr[0:D, :].rearrange("(c p) f -> p c f", p=128),
                    )
                    if use_bias:
                        nc.sync.dma_start(wsb[0:1, C, :], wdr[D : D + 1, :])

                for q4 in range(NQS):
                    sq = slice(q4 * 512, (q4 + 1) * 512)
                    xq = p1.tile([128, C + 1, 512], R32, tag="xq", bufs=2)
                    nc.sync.dma_start(
                        xq[:, 0:C, :],
                        xT[0:D, sq].rearrange("(c p) s -> p c s", p=128),
                    )
                    if use_bias:
                        nc.vector.memset(xq[0:1, C, :], 1.0)

                    for wsb, dst in ((wq_sb, QT_sb), (wk_sb, KT_sb)):
                        for t in range(NT):
                            qp = ps1.tile([128, 512], F32, tag="qk_ps", bufs=2)
                            for c in range(C):
                                nc.tensor.matmul(
                                    qp[:],
                                    lhsT=wsb[:, c, t * 128 : (t + 1) * 128],
                                    rhs=xq[:, c, :],
                                    start=(c == 0),
                                    stop=(c == C - 1 and not use_bias),
                                )
                            if use_bias:
                                nc.tensor.matmul(
                                    qp[:],
                                    lhsT=wsb[0:1, C, t * 128 : (t + 1) * 128],
                                    rhs=xq[0:1, C, :],
                                    start=False,
                                    stop=True,
                                )
                            nc.vector.tensor_copy(dst[:, t, sq], qp[:])

                    for sst in range(4):
                        st = q4 * 4 + sst
                        sl = slice(sst * 128, (sst + 1) * 128)
                        vp = ps1.tile([128, 512], F32, tag="v_ps", bufs=2)
                        for c in range(C):
                            nc.tensor.matmul(
                                vp[:],
                                lhsT=xq[:, c, sl],
                                rhs=wv_sb[:, c, :],
                                start=(c == 0),
                                stop=(c == C - 1 and not use_bias),
                            )
                        if use_bias:
                            nc.tensor.matmul(
                                vp[:],
                                lhsT=xq[0:1, C, sl],
                                rhs=wv_sb[0:1, C, :],
                                start=False,
                                stop=True,
                            )
                        nc.vector.tensor_copy(
                            V_sb[:, st, :, 0:HD],
                            vp.rearrange("p (h e) -> p h e", h=HLOC),
                        )

            # ---------------- Phase 2: per-head causal attention ----------------
            pvals = tc.alloc_tile_pool(name="pvals", bufs=1)
            vals_sb = pvals.tile([128, NFC, S], R32)
            with (
                tc.tile_pool(name="p2", bufs=1) as p2,
                tc.tile_pool(name="ps2", bufs=1, space="PSUM") as ps2,
            ):
                for h in range(HLOC):
                    t = h // 2
                    po = (h % 2) * HD
                    QT_h = QT_sb[po : po + HD, t, :]
                    KT_h = KT_sb[po : po + HD, t, :]
                    for qs in range(NQS):
                        sq = slice(qs * 512, (qs + 1) * 512)
                        out_ps = ps2.tile([HD + 1, 512], F32, tag="out_ps", bufs=2)
                        # j-groups of 4 k-chunks; diagonal group first so every
                        # group is full and the mask applies to group 0 only.
                        jbases = [4 * qs] + list(range(0, 4 * qs, 4))
                        first_j = 4 * qs
                        last_j = 4 * qs - 1 if qs > 0 else 4 * qs + 3
                        for gi, jb in enumerate(jbases):
                            sc_ps = ps2.tile([128, 4, 512], F32, tag="sc_ps", bufs=1)
                            for g in range(4):
                                j = jb + g
                                nc.tensor.matmul(
                                    sc_ps[:, g, :],
                                    lhsT=KT_h[:, j * 128 : (j + 1) * 128],
                                    rhs=QT_h[:, sq],
                                    start=True,
                                    stop=True,
                                )
                            ex = p2.tile([128, 4, 512], R32, tag="ex", bufs=2)
                            nc.scalar.activation(
                                ex[:], sc_ps[:], mybir.ActivationFunctionType.Exp,
                                scale=SCALE,
                            )
                            if gi == 0:
                                nc.vector.tensor_mul(ex[:], ex[:], mask_sb[:])
                            for g in range(4):
                                j = jb + g
                                nc.tensor.matmul(
                                    out_ps[:],
                                    lhsT=V_sb[:, j, h, :],
                                    rhs=ex[:, g, :],
                                    start=(j == first_j),
                                    stop=(j == last_j),
                                )
                        # normalize: rows 0..63 divided by row 64
                        r_row = p2.tile([1, 512], F32, tag="r_row", bufs=2)
                        nc.vector.tensor_copy(r_row[:], out_ps[HD : HD + 1, :])
                        nc.vector.reciprocal(r_row[:], r_row[:])
                        rc = p2.tile([HD, 512], F32, tag="rc", bufs=2)
                        nc.gpsimd.partition_broadcast(rc[:], r_row[:])
                        nc.vector.tensor_mul(
                            vals_sb[po : po + HD, t, sq], out_ps[0:HD, :], rc[:]
                        )

            # ---------------- Phase 3: output projection ----------------
            with (
                tc.tile_pool(name="p3", bufs=1) as p3,
                tc.tile_pool(name="ps3", bufs=1, space="PSUM") as ps3,
            ):
                wo_sb = p3.tile([128, NFC, D], R32)
                nc.sync.dma_start(wo_sb[:], wo.rearrange("(c p) n -> p c n", p=128))
                for st in range(NST):
                    sl = slice(st * 128, (st + 1) * 128)
                    yp = ps3.tile([128, D], F32, tag="y_ps", bufs=2)
                    for fc in range(NFC):
                        for nh in range(D // 512):
                            nc.tensor.matmul(
                                yp[:, nh * 512 : (nh + 1) * 512],
                                lhsT=vals_sb[:, fc, sl],
                                rhs=wo_sb[:, fc, nh * 512 : (nh + 1) * 512],
                                start=(fc == 0),
                                stop=(fc == NFC - 1),
                            )
                    yo = p3.tile([128, D], F32, tag="yo", bufs=2)
                    nc.vector.tensor_copy(yo[:], yp[:])
                    nc.sync.dma_start(y[sl, :], yo[:])
            pvals.release()

    nc.finalize()
    return nc


_NC_CACHE = {}


def _get_nc(use_bias: bool):
    if use_bias not in _NC_CACHE:
        _NC_CACHE[use_bias] = _build_nc(use_bias)
    return _NC_CACHE[use_bias]


def _make_mask() -> np.ndarray:
    # mask[k_local, g, q_local] = 1 if q_local >= g*128 + k_local (within the
    # diagonal 512-wide block; k-chunk offset g*128)
    kl = np.arange(128)[:, None, None]
    g = np.arange(4)[None, :, None]
    ql = np.arange(512)[None, None, :]
    return (ql >= g * 128 + kl).astype(np.float32).reshape(128, 2048)


def make_in_maps(x, W_qkv, b_qkv, W_out):
    use_bias = bool(np.any(b_qkv))
    mask = _make_mask()
    in_maps = []
    for core in range(8):
        b = core // 2
        hg = core % 2
        xt = np.ascontiguousarray(x[b].T)  # [D, S]
        q_cols = slice(hg * 512, (hg + 1) * 512)
        k_cols = slice(D + hg * 512, D + (hg + 1) * 512)
        v_cols = slice(2 * D + hg * 512, 2 * D + (hg + 1) * 512)
        wq_s = np.ascontiguousarray(W_qkv[:, q_cols])
        wk_s = np.ascontiguousarray(W_qkv[:, k_cols])
        wv_s = np.ascontiguousarray(W_qkv[:, v_cols])
        if use_bias:
            xt = np.concatenate([xt, np.ones((1, S), np.float32)], axis=0)
            wq_s = np.concatenate([wq_s, b_qkv[None, hg * 512 : (hg + 1) * 512]], axis=0)
            wk_s = np.concatenate(
                [wk_s, b_qkv[None, D + hg * 512 : D + (hg + 1) * 512]], axis=0
            )
            wv_s = np.concatenate(
                [wv_s, b_qkv[None, 2 * D + hg * 512 : 2 * D + (hg + 1) * 512]], axis=0
            )
        wo_s = np.ascontiguousarray(W_out[hg * 512 : (hg + 1) * 512, :])
        in_maps.append(
            {
                "xT": np.ascontiguousarray(xt, dtype=np.float32),
                "wq": wq_s.astype(np.float32),
                "wk": wk_s.astype(np.float32),
                "wv": wv_s.astype(np.float32),
                "wo": wo_s.astype(np.float32),
                "mask": mask,
            }
        )
    return in_maps, use_bias


def gather_output(results, b_out):
    y = np.empty((B, S, D), dtype=np.float32)
    for b in range(B):
        y[b] = results[2 * b]["y"] + results[2 * b + 1]["y"]
    if b_out is not None and np.any(b_out):
        y += b_out[None, None, :].astype(np.float32)
    return y


def kernel(x, W_qkv, b_qkv, W_out, b_out):
    x = np.asarray(x, dtype=np.float32)
    W_qkv = np.asarray(W_qkv, dtype=np.float32)
    b_qkv = np.asarray(b_qkv, dtype=np.float32)
    W_out = np.asarray(W_out, dtype=np.float32)
    b_out = np.asarray(b_out, dtype=np.float32)
    in_maps, use_bias = make_in_maps(x, W_qkv, b_qkv, W_out)
    nc = _get_nc(use_bias)
    res = run_bass_kernel_spmd(nc, in_maps, core_ids=list(range(8)))
    return gather_output(res.results, b_out)


# revision 9
# speedup vs baseline: 78.9982x; 78.9982x over previous
"""Causal self-attention (B=4, S=2048, D=1024, H=16) on 8 TRN2 NeuronCores.

Sharding: batch 4-way x head-group 2-way. Core c handles batch c//2 and
heads (c%2)*8 .. (c%2)*8+8. Each core computes its QKV projection slice,
per-head causal attention, and a partial output projection (W_out rows of
its heads); the host sums the two head-group partials per batch.

All TensorEngine matmuls run in float32r (fp32 storage, fast PE path).
"""

import os
import sys

for _p in ("/opt/trn_rl_repo", "/root/.axon_site/_ro/trn_rl_repo"):
    if os.path.isdir(_p) and _p not in sys.path:
        sys.path.insert(0, _p)

import numpy as np

import concourse.bass as bass  # noqa: E402
import concourse.tile as tile  # noqa: E402
from concourse import bacc, mybir  # noqa: E402
from concourse.bass_utils import run_bass_kernel_spmd  # noqa: E402

F32 = mybir.dt.float32
R32 = mybir.dt.float32r

B = 4
S = 2048
D = 1024
H = 16
HD = 64  # head dim
HLOC = 8  # heads per core
SCALE = HD ** -0.5

C = D // 128  # dm chunks (8)
NQS = S // 512  # q subtiles (4)
NST = S // 128  # s tiles (16)
NT = HLOC * HD // 128  # Q/K partition groups (4)
NFC = NT  # vals f-chunks (4)


def _build_nc(use_bias: bool, repeat: int = 1):
    from contextlib import ExitStack

    nc = bacc.Bacc(
        "TRN2",
        target_bir_lowering=False,
        debug=False,
        enable_asserts=True,
        num_devices=8,
    )

    dw = D + 1 if use_bias else D
    xT = nc.dram_tensor("xT", [dw, S], R32, kind="ExternalInput")
    wq = nc.dram_tensor("wq", [dw, 512], R32, kind="ExternalInput")
    wk = nc.dram_tensor("wk", [dw, 512], R32, kind="ExternalInput")
    wv = nc.dram_tensor("wv", [dw, 512], R32, kind="ExternalInput")
    wo = nc.dram_tensor("wo", [512, D], R32, kind="ExternalInput")
    mask = nc.dram_tensor("mask", [128, 4 * 512], R32, kind="ExternalInput")
    y = nc.dram_tensor("y", [S, D], F32, kind="ExternalOutput")

    with tile.TileContext(nc) as tc, ExitStack() as es:
        if repeat > 1:
            es.enter_context(tc.For_i(0, repeat, 1))
        with tc.tile_pool(name="persist", bufs=1) as persist:
            mask_sb = persist.tile([128, 4, 512], R32)
            nc.sync.dma_start(mask_sb[:], mask.rearrange("p (g q) -> p g q", g=4))
            QT_sb = persist.tile([128, NT, S], R32)
            KT_sb = persist.tile([128, NT, S], R32)
            V_sb = persist.tile([128, NST, HLOC, HD + 1], R32)
            nc.vector.memset(
                V_sb.rearrange("p a b c -> p (a b c)").bitcast(F32), 1.0
            )

            # ---------------- Phase 1: QKV^T projection ----------------
            with (
                tc.tile_pool(name="p1", bufs=1) as p1,
                tc.tile_pool(name="ps1", bufs=1, space="PSUM") as ps1,
            ):
                wq_sb = p1.tile([128, C + 1, 512], R32)
                wk_sb = p1.tile([128, C + 1, 512], R32)
                wv_sb = p1.tile([128, C + 1, 512], R32)
                for wsb, wdr in ((wq_sb, wq), (wk_sb, wk), (wv_sb, wv)):
                    nc.sync.dma_start(
                        wsb[:, 0:C, :],
                        wdr[0:D, :].rearrange("(c p) f -> p c f", p=128),
                    )
                    if use_bias:
                        nc.sync.dma_start(wsb[0:1, C, :], wdr[D : D + 1, :])

                for q4 in range(NQS):
                    sq = slice(q4 * 512, (q4 + 1) * 512)
                    xq = p1.tile([128, C + 1, 512], R32, tag="xq", bufs=2)
                    nc.sync.dma_start(
                        xq[:, 0:C, :],
                        xT[0:D, sq].rearrange("(c p) s -> p c s", p=128),
                    )
                    if use_bias:
                        nc.vector.memset(xq[0:1, C, :], 1.0)

                    for wsb, dst in ((wq_sb, QT_sb), (wk_sb, KT_sb)):
                        for t in range(NT):
                            qp = ps1.tile([128, 512], F32, tag="qk_ps", bufs=2)
                            for c in range(C):
                                nc.tensor.matmul(
                                    qp[:],
                                    lhsT=wsb[:, c, t * 128 : (t + 1) * 128],
                                    rhs=xq[:, c, :],
                                    start=(c == 0),
                                    stop=(c == C - 1 and not use_bias),
                                )
                            if use_bias:
                                nc.tensor.matmul(
                                    qp[:],
                                    lhsT=wsb[0:1, C, t * 128 : (t + 1) * 128],
                                    rhs=xq[0:1, C, :],
                                    start=False,
                                    stop=True,
                                )
                            nc.vector.tensor_copy(dst[:, t, sq], qp[:])

                    for sst in range(4):
                        st = q4 * 4 + sst
                        sl = slice(sst * 128, (sst + 1) * 128)
                        vp = ps1.tile([128, 512], F32, tag="v_ps", bufs=2)
                        for c in range(C):
                            nc.tensor.matmul(
                                vp[:],
                                lhsT=xq[:, c, sl],
                                rhs=wv_sb[:, c, :],
                                start=(c == 0),
                                stop=(c == C - 1 and not use_bias),
                            )
                        if use_bias:
                            nc.tensor.matmul(
                                vp[:],
                                lhsT=xq[0:1, C, sl],
                                rhs=wv_sb[0:1, C, :],
                                start=False,
                                stop=True,
                            )
                        nc.vector.tensor_copy(
                            V_sb[:, st, :, 0:HD],
                            vp.rearrange("p (h e) -> p h e", h=HLOC),
                        )

            # ---------------- Phase 2: per-head causal attention ----------------
            pvals = tc.alloc_tile_pool(name="pvals", bufs=1)
            vals_sb = pvals.tile([128, NFC, S], R32)
            with (
                tc.tile_pool(name="p2", bufs=1) as p2,
                tc.tile_pool(name="ps2", bufs=1, space="PSUM") as ps2,
            ):
                for h in range(HLOC):
                    t = h // 2
                    po = (h % 2) * HD
                    QT_h = QT_sb[po : po + HD, t, :]
                    KT_h = KT_sb[po : po + HD, t, :]
                    for qs in range(NQS):
                        sq = slice(qs * 512, (qs + 1) * 512)
                        out_ps = ps2.tile([HD + 1, 512], F32, tag="out_ps", bufs=2)
                        # j-groups of 4 k-chunks; diagonal group first so every
                        # group is full and the mask applies to group 0 only.
                        jbases = [4 * qs] + list(range(0, 4 * qs, 4))
                        first_j = 4 * qs
                        last_j = 4 * qs - 1 if qs > 0 else 4 * qs + 3
                        for gi, jb in enumerate(jbases):
                            sc_ps = ps2.tile([128, 4, 512], F32, tag="sc_ps", bufs=1)
                            for g in range(4):
                                j = jb + g
                                nc.tensor.matmul(
                                    sc_ps[:, g, :],
                                    lhsT=KT_h[:, j * 128 : (j + 1) * 128],
                                    rhs=QT_h[:, sq],
                                    start=True,
                                    stop=True,
                                )
                            ex = p2.tile([128, 4, 512], R32, tag="ex", bufs=2)
                            nc.scalar.activation(
                                ex[:], sc_ps[:], mybir.ActivationFunctionType.Exp,
                                scale=SCALE,
                            )
                            if gi == 0:
                                nc.vector.tensor_mul(ex[:], ex[:], mask_sb[:])
                            for g in range(4):
                                j = jb + g
                                nc.tensor.matmul(
                                    out_ps[:],
                                    lhsT=V_sb[:, j, h, :],
                                    rhs=ex[:, g, :],
                                    start=(j == first_j),
                                    stop=(j == last_j),
                                )
                        # normalize: rows 0..63 divided by row 64
                        r_row = p2.tile([1, 512], F32, tag="r_row", bufs=2)
                        nc.vector.tensor_copy(r_row[:], out_ps[HD : HD + 1, :])
                        nc.vector.reciprocal(r_row[:], r_row[:])
                        rc = p2.tile([HD, 512], F32, tag="rc", bufs=2)
                        nc.gpsimd.partition_broadcast(rc[:], r_row[:])
                        nc.vector.tensor_mul(
                            vals_sb[po : po + HD, t, sq], out_ps[0:HD, :], rc[:]
                        )

            # ---------------- Phase 3: output projection ----------------
            with (
                tc.tile_pool(name="p3", bufs=1) as p3,
                tc.tile_pool(name="ps3", bufs=1, space="PSUM") as ps3,
            ):
                wo_sb = p3.tile([128, NFC, D], R32)
                nc.sync.dma_start(wo_sb[:], wo.rearrange("(c p) n -> p c n", p=128))
                for st in range(NST):
                    sl = slice(st * 128, (st + 1) * 128)
                    yp = ps3.tile([128, D], F32, tag="y_ps", bufs=2)
                    for fc in range(NFC):
                        for nh in range(D // 512):
                            nc.tensor.matmul(
                                yp[:, nh * 512 : (nh + 1) * 512],
                                lhsT=vals_sb[:, fc, sl],
                                rhs=wo_sb[:, fc, nh * 512 : (nh + 1) * 512],
                                start=(fc == 0),
                                stop=(fc == NFC - 1),
                            )
                    yo = p3.tile([128, D], F32, tag="yo", bufs=2)
                    nc.vector.tensor_copy(yo[:], yp[:])
                    nc.sync.dma_start(y[sl, :], yo[:])
            pvals.release()

    nc.finalize()
    return nc


_NC_CACHE = {}


def _get_nc(use_bias: bool, repeat: int = 1):
    key = (use_bias, repeat)
    if key not in _NC_CACHE:
        _NC_CACHE[key] = _build_nc(use_bias, repeat)
    return _NC_CACHE[key]


def _make_mask() -> np.ndarray:
    # mask[k_local, g, q_local] = 1 if q_local >= g*128 + k_local (within the
    # diagonal 512-wide block; k-chunk offset g*128)
    kl = np.arange(128)[:, None, None]
    g = np.arange(4)[None, :, None]
    ql = np.arange(512)[None, None, :]
    return (ql >= g * 128 + kl).astype(np.float32).reshape(128, 2048)


def make_in_maps(x, W_qkv, b_qkv, W_out):
    use_bias = bool(np.any(b_qkv))
    mask = _make_mask()
    in_maps = []
    for core in range(8):
        b = core // 2
        hg = core % 2
        xt = np.ascontiguousarray(x[b].T)  # [D, S]
        q_cols = slice(hg * 512, (hg + 1) * 512)
        k_cols = slice(D + hg * 512, D + (hg + 1) * 512)
        v_cols = slice(2 * D + hg * 512, 2 * D + (hg + 1) * 512)
        wq_s = np.ascontiguousarray(W_qkv[:, q_cols])
        wk_s = np.ascontiguousarray(W_qkv[:, k_cols])
        wv_s = np.ascontiguousarray(W_qkv[:, v_cols])
        if use_bias:
            xt = np.concatenate([xt, np.ones((1, S), np.float32)], axis=0)
            wq_s = np.concatenate([wq_s, b_qkv[None, hg * 512 : (hg + 1) * 512]], axis=0)
            wk_s = np.concatenate(
                [wk_s, b_qkv[None, D + hg * 512 : D + (hg + 1) * 512]], axis=0
            )
            wv_s = np.concatenate(
                [wv_s, b_qkv[None, 2 * D + hg * 512 : 2 * D + (hg + 1) * 512]], axis=0
            )
        wo_s = np.ascontiguousarray(W_out[hg * 512 : (hg + 1) * 512, :])
        in_maps.append(
            {
                "xT": np.ascontiguousarray(xt, dtype=np.float32),
                "wq": wq_s.astype(np.float32),
                "wk": wk_s.astype(np.float32),
                "wv": wv_s.astype(np.float32),
                "wo": wo_s.astype(np.float32),
                "mask": mask,
            }
        )
    return in_maps, use_bias


def gather_output(results, b_out):
    y = np.empty((B, S, D), dtype=np.float32)
    for b in range(B):
        y[b] = results[2 * b]["y"] + results[2 * b + 1]["y"]
    if b_out is not None and np.any(b_out):
        y += b_out[None, None, :].astype(np.float32)
    return y


def kernel(x, W_qkv, b_qkv, W_out, b_out):
    x = np.asarray(x, dtype=np.float32)
    W_qkv = np.asarray(W_qkv, dtype=np.float32)
    b_qkv = np.asarray(b_qkv, dtype=np.float32)
    W_out = np.asarray(W_out, dtype=np.float32)
    b_out = np.asarray(b_out, dtype=np.float32)
    in_maps, use_bias = make_in_maps(x, W_qkv, b_qkv, W_out)
    nc = _get_nc(use_bias)
    res = run_bass_kernel_spmd(nc, in_maps, core_ids=list(range(8)))
    return gather_output(res.results, b_out)


# revision 14
# speedup vs baseline: 95.0934x; 1.2037x over previous
"""Causal self-attention (B=4, S=2048, D=1024, H=16) on 8 TRN2 NeuronCores.

Sharding: batch 4-way x head-group 2-way. Core c handles batch c//2 and
heads (c%2)*8 .. (c%2)*8+8. Each core computes its QKV projection slice,
per-head causal attention, and a partial output projection (W_out rows of
its heads); the host sums the two head-group partials per batch.

All TensorEngine matmuls run in float32r (fp32 storage, fast PE path).
"""

import os
import sys

for _p in ("/opt/trn_rl_repo", "/root/.axon_site/_ro/trn_rl_repo"):
    if os.path.isdir(_p) and _p not in sys.path:
        sys.path.insert(0, _p)

import numpy as np

import concourse.bass as bass  # noqa: E402
import concourse.tile as tile  # noqa: E402
from concourse import bacc, mybir  # noqa: E402
from concourse.bass_utils import run_bass_kernel_spmd  # noqa: E402

F32 = mybir.dt.float32
R32 = mybir.dt.float32r

B = 4
S = 2048
D = 1024
H = 16
HD = 64  # head dim
HLOC = 8  # heads per core
SCALE = HD ** -0.5

C = D // 128  # dm chunks (8)
NQS = S // 512  # q subtiles (4)
NST = S // 128  # s tiles (16)
NT = HLOC * HD // 128  # Q/K partition groups (4)
NFC = NT  # vals f-chunks (4)


def _build_nc(use_bias: bool, repeat: int = 1):
    from contextlib import ExitStack

    nc = bacc.Bacc(
        "TRN2",
        target_bir_lowering=False,
        debug=False,
        enable_asserts=True,
        num_devices=8,
    )

    dw = D + 1 if use_bias else D
    xT = nc.dram_tensor("xT", [dw, S], R32, kind="ExternalInput")
    wq = nc.dram_tensor("wq", [dw, 512], R32, kind="ExternalInput")
    wk = nc.dram_tensor("wk", [dw, 512], R32, kind="ExternalInput")
    wv = nc.dram_tensor("wv", [dw, 512], R32, kind="ExternalInput")
    wo = nc.dram_tensor("wo", [512, D], R32, kind="ExternalInput")
    mask = nc.dram_tensor("mask", [128, 128], R32, kind="ExternalInput")
    y = nc.dram_tensor("y", [S, D], F32, kind="ExternalOutput")

    with tile.TileContext(nc) as tc, ExitStack() as es:
        if repeat > 1:
            es.enter_context(tc.For_i(0, repeat, 1))
        with tc.tile_pool(name="persist", bufs=1) as persist:
            mask_sb = persist.tile([128, 128], R32)
            nc.sync.dma_start(mask_sb[:], mask[:])
            QT_sb = persist.tile([128, NT, S], R32)
            KT_sb = persist.tile([128, NT, S], R32)
            V_sb = persist.tile([128, NST, HLOC, HD + 1], R32)
            nc.vector.memset(
                V_sb.rearrange("p a b c -> p (a b c)").bitcast(F32), 1.0
            )

            # ---------------- Phase 1: QKV^T projection ----------------
            with (
                tc.tile_pool(name="p1", bufs=1) as p1,
                tc.tile_pool(name="ps1", bufs=1, space="PSUM") as ps1,
            ):
                wq_sb = p1.tile([128, C + 1, 512], R32)
                wk_sb = p1.tile([128, C + 1, 512], R32)
                wv_sb = p1.tile([128, C + 1, 512], R32)
                for wsb, wdr in ((wq_sb, wq), (wk_sb, wk), (wv_sb, wv)):
                    nc.sync.dma_start(
                        wsb[:, 0:C, :],
                        wdr[0:D, :].rearrange("(c p) f -> p c f", p=128),
                    )
                    if use_bias:
                        nc.sync.dma_start(wsb[0:1, C, :], wdr[D : D + 1, :])

                for q4 in range(NQS):
                    sq = slice(q4 * 512, (q4 + 1) * 512)
                    xq = p1.tile([128, C + 1, 512], R32, tag="xq", bufs=2)
                    nc.sync.dma_start(
                        xq[:, 0:C, :],
                        xT[0:D, sq].rearrange("(c p) s -> p c s", p=128),
                    )
                    if use_bias:
                        nc.vector.memset(xq[0:1, C, :], 1.0)

                    for wsb, dst in ((wq_sb, QT_sb), (wk_sb, KT_sb)):
                        for t in range(NT):
                            qp = ps1.tile([128, 512], F32, tag="qk_ps", bufs=2)
                            for c in range(C):
                                nc.tensor.matmul(
                                    qp[:],
                                    lhsT=wsb[:, c, t * 128 : (t + 1) * 128],
                                    rhs=xq[:, c, :],
                                    start=(c == 0),
                                    stop=(c == C - 1 and not use_bias),
                                )
                            if use_bias:
                                nc.tensor.matmul(
                                    qp[:],
                                    lhsT=wsb[0:1, C, t * 128 : (t + 1) * 128],
                                    rhs=xq[0:1, C, :],
                                    start=False,
                                    stop=True,
                                )
                            nc.any.tensor_copy(dst[:, t, sq], qp[:])

                    for sst in range(4):
                        st = q4 * 4 + sst
                        sl = slice(sst * 128, (sst + 1) * 128)
                        vp = ps1.tile([128, 512], F32, tag="v_ps", bufs=2)
                        for c in range(C):
                            nc.tensor.matmul(
                                vp[:],
                                lhsT=xq[:, c, sl],
                                rhs=wv_sb[:, c, :],
                                start=(c == 0),
                                stop=(c == C - 1 and not use_bias),
                            )
                        if use_bias:
                            nc.tensor.matmul(
                                vp[:],
                                lhsT=xq[0:1, C, sl],
                                rhs=wv_sb[0:1, C, :],
                                start=False,
                                stop=True,
                            )
                        nc.any.tensor_copy(
                            V_sb[:, st, :, 0:HD],
                            vp.rearrange("p (h e) -> p h e", h=HLOC),
                        )

            # ---------------- Phase 2: per-head causal attention ----------------
            pvals = tc.alloc_tile_pool(name="pvals", bufs=1)
            vals_sb = pvals.tile([128, NFC, S], R32)
            with (
                tc.tile_pool(name="p2", bufs=1) as p2,
                tc.tile_pool(name="ps2", bufs=1, space="PSUM") as ps2,
            ):
                for h in range(HLOC):
                    t = h // 2
                    po = (h % 2) * HD
                    QT_h = QT_sb[po : po + HD, t, :]
                    KT_h = KT_sb[po : po + HD, t, :]
                    for qs in range(NQS):
                        sq = slice(qs * 512, (qs + 1) * 512)
                        out_ps = ps2.tile([HD + 1, 512], F32, tag="out_ps", bufs=2)
                        # diagonal k-chunks first, then 0..4qs-1; groups of 3
                        # so scores-PSUM (3 banks x 2 bufs) + out (2) fit PSUM.
                        js = list(range(4 * qs, 4 * qs + 4)) + list(range(0, 4 * qs))
                        npos = len(js)
                        for gb in range(0, npos, 3):
                            chunk = js[gb : gb + 3]
                            ng = len(chunk)
                            sc_ps = ps2.tile([128, 3, 512], F32, tag="sc_ps", bufs=2)
                            for gg, j in enumerate(chunk):
                                nc.tensor.matmul(
                                    sc_ps[:, gg, :],
                                    lhsT=KT_h[:, j * 128 : (j + 1) * 128],
                                    rhs=QT_h[:, sq],
                                    start=True,
                                    stop=True,
                                )
                            ex = p2.tile([128, 3, 512], R32, tag="ex", bufs=3)
                            nc.scalar.activation(
                                ex[:, :ng, :], sc_ps[:, :ng, :],
                                mybir.ActivationFunctionType.Exp, scale=SCALE,
                            )
                            for gg, j in enumerate(chunk):
                                dg = j - 4 * qs
                                if 0 <= dg < 4:
                                    # diagonal chunk: mask the 128-wide mixed
                                    # band; columns < 128*dg are excluded from
                                    # the PV moving range below (exact zeros).
                                    nc.vector.tensor_mul(
                                        ex[:, gg, dg * 128 : (dg + 1) * 128],
                                        ex[:, gg, dg * 128 : (dg + 1) * 128],
                                        mask_sb[:],
                                    )
                            for gg, j in enumerate(chunk):
                                dg = j - 4 * qs
                                qlo = dg * 128 if 0 <= dg < 4 else 0
                                nc.tensor.matmul(
                                    out_ps[:, qlo:512],
                                    lhsT=V_sb[:, j, h, :],
                                    rhs=ex[:, gg, qlo:512],
                                    start=(gb == 0 and gg == 0),
                                    stop=(gb + gg == npos - 1),
                                )
                        # normalize: rows 0..63 divided by row 64
                        r_row = p2.tile([1, 512], F32, tag="r_row", bufs=2)
                        nc.vector.tensor_copy(r_row[:], out_ps[HD : HD + 1, :])
                        nc.vector.reciprocal(r_row[:], r_row[:])
                        rc = p2.tile([HD, 512], F32, tag="rc", bufs=2)
                        nc.gpsimd.partition_broadcast(rc[:], r_row[:])
                        nc.vector.tensor_mul(
                            vals_sb[po : po + HD, t, sq], out_ps[0:HD, :], rc[:]
                        )

            # ---------------- Phase 3: output projection ----------------
            with (
                tc.tile_pool(name="p3", bufs=1) as p3,
                tc.tile_pool(name="ps3", bufs=1, space="PSUM") as ps3,
            ):
                wo_sb = p3.tile([128, NFC, D], R32)
                nc.sync.dma_start(wo_sb[:], wo.rearrange("(c p) n -> p c n", p=128))
                for st in range(NST):
                    sl = slice(st * 128, (st + 1) * 128)
                    yp = ps3.tile([128, D], F32, tag="y_ps", bufs=2)
                    for fc in range(NFC):
                        for nh in range(D // 512):
                            nc.tensor.matmul(
                                yp[:, nh * 512 : (nh + 1) * 512],
                                lhsT=vals_sb[:, fc, sl],
                                rhs=wo_sb[:, fc, nh * 512 : (nh + 1) * 512],
                                start=(fc == 0),
                                stop=(fc == NFC - 1),
                            )
                    yo = p3.tile([128, D], F32, tag="yo", bufs=2)
                    nc.any.tensor_copy(yo[:], yp[:])
                    nc.sync.dma_start(y[sl, :], yo[:])
            pvals.release()

    nc.finalize()
    return nc


_NC_CACHE = {}


def _get_nc(use_bias: bool, repeat: int = 1):
    key = (use_bias, repeat)
    if key not in _NC_CACHE:
        _NC_CACHE[key] = _build_nc(use_bias, repeat)
    return _NC_CACHE[key]


def _make_mask() -> np.ndarray:
    # upper-tri-inclusive band mask: keep[k_local, q_local] = q_local >= k_local
    kl = np.arange(128)[:, None]
    ql = np.arange(128)[None, :]
    return (ql >= kl).astype(np.float32)


def make_in_maps(x, W_qkv, b_qkv, W_out):
    use_bias = bool(np.any(b_qkv))
    mask = _make_mask()
    in_maps = []
    for core in range(8):
        b = core // 2
        hg = core % 2
        xt = np.ascontiguousarray(x[b].T)  # [D, S]
        q_cols = slice(hg * 512, (hg + 1) * 512)
        k_cols = slice(D + hg * 512, D + (hg + 1) * 512)
        v_cols = slice(2 * D + hg * 512, 2 * D + (hg + 1) * 512)
        wq_s = np.ascontiguousarray(W_qkv[:, q_cols])
        wk_s = np.ascontiguousarray(W_qkv[:, k_cols])
        wv_s = np.ascontiguousarray(W_qkv[:, v_cols])
        if use_bias:
            xt = np.concatenate([xt, np.ones((1, S), np.float32)], axis=0)
            wq_s = np.concatenate([wq_s, b_qkv[None, hg * 512 : (hg + 1) * 512]], axis=0)
            wk_s = np.concatenate(
                [wk_s, b_qkv[None, D + hg * 512 : D + (hg + 1) * 512]], axis=0
            )
            wv_s = np.concatenate(
                [wv_s, b_qkv[None, 2 * D + hg * 512 : 2 * D + (hg + 1) * 512]], axis=0
            )
        wo_s = np.ascontiguousarray(W_out[hg * 512 : (hg + 1) * 512, :])
        in_maps.append(
            {
                "xT": np.ascontiguousarray(xt, dtype=np.float32),
                "wq": wq_s.astype(np.float32),
                "wk": wk_s.astype(np.float32),
                "wv": wv_s.astype(np.float32),
                "wo": wo_s.astype(np.float32),
                "mask": mask,
            }
        )
    return in_maps, use_bias


def gather_output(results, b_out):
    y = np.empty((B, S, D), dtype=np.float32)
    for b in range(B):
        y[b] = results[2 * b]["y"] + results[2 * b + 1]["y"]
    if b_out is not None and np.any(b_out):
        y += b_out[None, None, :].astype(np.float32)
    return y


def kernel(x, W_qkv, b_qkv, W_out, b_out):
    x = np.asarray(x, dtype=np.float32)
    W_qkv = np.asarray(W_qkv, dtype=np.float32)
    b_qkv = np.asarray(b_qkv, dtype=np.float32)
    W_out = np.asarray(W_out, dtype=np.float32)
    b_out = np.asarray(b_out, dtype=np.float32)
    in_maps, use_bias = make_in_maps(x, W_qkv, b_qkv, W_out)
    nc = _get_nc(use_bias)
    res = run_bass_kernel_spmd(nc, in_maps, core_ids=list(range(8)))
    return gather_output(res.results, b_out)


# revision 17
# speedup vs baseline: 132.6855x; 1.3953x over previous
"""Causal self-attention (B=4, S=2048, D=1024, H=16) on 8 TRN2 NeuronCores.

Sharding: batch 4-way x head-group 2-way. Core c handles batch c//2 and
heads (c%2)*8 .. (c%2)*8+8. Each core computes its QKV projection slice,
per-head causal attention, and a partial output projection (W_out rows of
its heads); the host sums the two head-group partials per batch.

All TensorEngine matmuls run in float32r (fp32 storage, fast PE path).
"""

import os
import sys

for _p in ("/opt/trn_rl_repo", "/root/.axon_site/_ro/trn_rl_repo"):
    if os.path.isdir(_p) and _p not in sys.path:
        sys.path.insert(0, _p)

import numpy as np

import concourse.bass as bass  # noqa: E402
import concourse.tile as tile  # noqa: E402
from concourse import bacc, mybir  # noqa: E402
from concourse.bass_utils import run_bass_kernel_spmd  # noqa: E402

F32 = mybir.dt.float32
R32 = mybir.dt.float32r

B = 4
S = 2048
D = 1024
H = 16
HD = 64  # head dim
HLOC = 8  # heads per core
SCALE = HD ** -0.5

C = D // 128  # dm chunks (8)
NQS = S // 512  # q subtiles (4)
NST = S // 128  # s tiles (16)
NT = HLOC * HD // 128  # Q/K partition groups (4)
NFC = NT  # vals f-chunks (4)


def _build_nc(use_bias: bool, repeat: int = 1):
    from contextlib import ExitStack

    nc = bacc.Bacc(
        "TRN2",
        target_bir_lowering=False,
        debug=False,
        enable_asserts=True,
        num_devices=8,
    )

    dw = D + 1 if use_bias else D
    xT = nc.dram_tensor("xT", [dw, S], R32, kind="ExternalInput")
    wq = nc.dram_tensor("wq", [dw, 512], R32, kind="ExternalInput")
    wk = nc.dram_tensor("wk", [dw, 512], R32, kind="ExternalInput")
    wv = nc.dram_tensor("wv", [dw, 512], R32, kind="ExternalInput")
    wo = nc.dram_tensor("wo", [512, D], R32, kind="ExternalInput")
    mask = nc.dram_tensor("mask", [128, 128], R32, kind="ExternalInput")
    y = nc.dram_tensor("y", [S, D], F32, kind="ExternalOutput")

    with tile.TileContext(nc) as tc, ExitStack() as es:
        if repeat > 1:
            es.enter_context(tc.For_i(0, repeat, 1))
        with tc.tile_pool(name="persist", bufs=1) as persist:
            mask_sb = persist.tile([128, 128], R32)
            nc.sync.dma_start(mask_sb[:], mask[:])
            QT_sb = persist.tile([128, NT, S], R32)
            KT_sb = persist.tile([128, NT, S], R32)
            V_sb = persist.tile([128, NST, HLOC, HD + 1], R32)
            nc.vector.memset(
                V_sb.rearrange("p a b c -> p (a b c)").bitcast(F32), 1.0
            )

            # ---------------- Phase 1: QKV^T projection ----------------
            with (
                tc.tile_pool(name="p1", bufs=1) as p1,
                tc.tile_pool(name="ps1", bufs=1, space="PSUM") as ps1,
            ):
                wq_sb = p1.tile([128, C + 1, 512], R32)
                wk_sb = p1.tile([128, C + 1, 512], R32)
                wv_sb = p1.tile([128, C + 1, 512], R32)
                for wsb, wdr in ((wq_sb, wq), (wk_sb, wk), (wv_sb, wv)):
                    nc.sync.dma_start(
                        wsb[:, 0:C, :],
                        wdr[0:D, :].rearrange("(c p) f -> p c f", p=128),
                    )
                    if use_bias:
                        nc.sync.dma_start(wsb[0:1, C, :], wdr[D : D + 1, :])

                for q4 in range(NQS):
                    sq = slice(q4 * 512, (q4 + 1) * 512)
                    xq = p1.tile([128, C + 1, 512], R32, tag="xq", bufs=2)
                    nc.sync.dma_start(
                        xq[:, 0:C, :],
                        xT[0:D, sq].rearrange("(c p) s -> p c s", p=128),
                    )
                    if use_bias:
                        nc.vector.memset(xq[0:1, C, :], 1.0)

                    for wsb, dst in ((wq_sb, QT_sb), (wk_sb, KT_sb)):
                        for t in range(NT):
                            qp = ps1.tile([128, 512], F32, tag="qk_ps", bufs=2)
                            for c in range(C):
                                nc.tensor.matmul(
                                    qp[:],
                                    lhsT=wsb[:, c, t * 128 : (t + 1) * 128],
                                    rhs=xq[:, c, :],
                                    start=(c == 0),
                                    stop=(c == C - 1 and not use_bias),
                                )
                            if use_bias:
                                nc.tensor.matmul(
                                    qp[:],
                                    lhsT=wsb[0:1, C, t * 128 : (t + 1) * 128],
                                    rhs=xq[0:1, C, :],
                                    start=False,
                                    stop=True,
                                )
                            nc.any.tensor_copy(dst[:, t, sq], qp[:])

                    for sst in range(4):
                        st = q4 * 4 + sst
                        sl = slice(sst * 128, (sst + 1) * 128)
                        vp = ps1.tile([128, 512], F32, tag="v_ps", bufs=2)
                        for c in range(C):
                            nc.tensor.matmul(
                                vp[:],
                                lhsT=xq[:, c, sl],
                                rhs=wv_sb[:, c, :],
                                start=(c == 0),
                                stop=(c == C - 1 and not use_bias),
                            )
                        if use_bias:
                            nc.tensor.matmul(
                                vp[:],
                                lhsT=xq[0:1, C, sl],
                                rhs=wv_sb[0:1, C, :],
                                start=False,
                                stop=True,
                            )
                        nc.any.tensor_copy(
                            V_sb[:, st, :, 0:HD],
                            vp.rearrange("p (h e) -> p h e", h=HLOC),
                        )

            # ---------------- Phase 2: per-head causal attention ----------------
            pvals = tc.alloc_tile_pool(name="pvals", bufs=1)
            vals_sb = pvals.tile([128, NFC, S], R32)
            with (
                tc.tile_pool(name="p2", bufs=1) as p2,
                tc.tile_pool(name="ps2", bufs=1, space="PSUM") as ps2,
            ):
                for t in range(NT):
                    # head pair (2t, 2t+1): even head on partitions 0-63, odd
                    # on 64-127 -> adjacent K=64 score matmuls target disjoint
                    # PE row groups and run concurrently.
                    for qs in range(NQS):
                        sq = slice(qs * 512, (qs + 1) * 512)
                        outs = [
                            ps2.tile([HD + 1, 512], F32, tag=f"out{p}", bufs=1,
                                     name=f"out{p}")
                            for p in range(2)
                        ]
                        # diagonal k-chunks first, then 0..4qs-1; groups of 3:
                        # sc PSUM (3 banks x 2 heads) + out (2) fill all 8 banks.
                        js = list(range(4 * qs, 4 * qs + 4)) + list(range(0, 4 * qs))
                        npos = len(js)
                        for gb in range(0, npos, 3):
                            chunk = js[gb : gb + 3]
                            ng = len(chunk)
                            scs = [
                                ps2.tile([128, 3, 512], F32, tag=f"sc{p}", bufs=1,
                                         name=f"sc{p}")
                                for p in range(2)
                            ]
                            for gg, j in enumerate(chunk):
                                for p in range(2):
                                    po = p * HD
                                    nc.tensor.matmul(
                                        scs[p][:, gg, :],
                                        lhsT=KT_sb[po : po + HD, t,
                                                   j * 128 : (j + 1) * 128],
                                        rhs=QT_sb[po : po + HD, t, sq],
                                        start=True,
                                        stop=True,
                                    )
                            exs = []
                            for p in range(2):
                                ex = p2.tile([128, 3, 512], R32, tag=f"ex{p}",
                                             bufs=2, name=f"ex{p}")
                                exs.append(ex)
                                nc.scalar.activation(
                                    ex[:, :ng, :], scs[p][:, :ng, :],
                                    mybir.ActivationFunctionType.Exp, scale=SCALE,
                                )
                                for gg, j in enumerate(chunk):
                                    dg = j - 4 * qs
                                    if 0 <= dg < 4:
                                        # mask only the 128-wide mixed band;
                                        # columns < 128*dg are excluded from the
                                        # PV moving range below (exact zeros).
                                        nc.vector.tensor_mul(
                                            ex[:, gg, dg * 128 : (dg + 1) * 128],
                                            ex[:, gg, dg * 128 : (dg + 1) * 128],
                                            mask_sb[:],
                                        )
                            for gg, j in enumerate(chunk):
                                dg = j - 4 * qs
                                qlo = dg * 128 if 0 <= dg < 4 else 0
                                for p in range(2):
                                    nc.tensor.matmul(
                                        outs[p][:, qlo:512],
                                        lhsT=V_sb[:, j, 2 * t + p, :],
                                        rhs=exs[p][:, gg, qlo:512],
                                        start=(gb == 0 and gg == 0),
                                        stop=(gb + gg == npos - 1),
                                    )
                        # normalize: rows 0..63 divided by row 64
                        for p in range(2):
                            po = p * HD
                            r_row = p2.tile([1, 512], F32, tag=f"r_row{p}", bufs=2,
                                            name=f"r_row{p}")
                            nc.vector.tensor_copy(r_row[:], outs[p][HD : HD + 1, :])
                            nc.vector.reciprocal(r_row[:], r_row[:])
                            rc = p2.tile([HD, 512], F32, tag=f"rc{p}", bufs=2,
                                         name=f"rc{p}")
                            nc.gpsimd.partition_broadcast(rc[:], r_row[:])
                            nc.vector.tensor_mul(
                                vals_sb[po : po + HD, t, sq], outs[p][0:HD, :], rc[:]
                            )

            # ---------------- Phase 3: output projection ----------------
            with (
                tc.tile_pool(name="p3", bufs=1) as p3,
                tc.tile_pool(name="ps3", bufs=1, space="PSUM") as ps3,
            ):
                wo_sb = p3.tile([128, NFC, D], R32)
                nc.sync.dma_start(wo_sb[:], wo.rearrange("(c p) n -> p c n", p=128))
                for st in range(NST):
                    sl = slice(st * 128, (st + 1) * 128)
                    yp = ps3.tile([128, D], F32, tag="y_ps", bufs=2)
                    for fc in range(NFC):
                        for nh in range(D // 512):
                            nc.tensor.matmul(
                                yp[:, nh * 512 : (nh + 1) * 512],
                                lhsT=vals_sb[:, fc, sl],
                                rhs=wo_sb[:, fc, nh * 512 : (nh + 1) * 512],
                                start=(fc == 0),
                                stop=(fc == NFC - 1),
                            )
                    yo = p3.tile([128, D], F32, tag="yo", bufs=2)
                    nc.any.tensor_copy(yo[:], yp[:])
                    nc.gpsimd.dma_start(y[sl, :], yo[:])
            pvals.release()

    nc.finalize()
    return nc


_NC_CACHE = {}


def _get_nc(use_bias: bool, repeat: int = 1):
    key = (use_bias, repeat)
    if key not in _NC_CACHE:
        _NC_CACHE[key] = _build_nc(use_bias, repeat)
    return _NC_CACHE[key]


def _make_mask() -> np.ndarray:
    # upper-tri-inclusive band mask: keep[k_local, q_local] = q_local >= k_local
    kl = np.arange(128)[:, None]
    ql = np.arange(128)[None, :]
    return (ql >= kl).astype(np.float32)


def make_in_maps(x, W_qkv, b_qkv, W_out):
    use_bias = bool(np.any(b_qkv))
    mask = _make_mask()
    in_maps = []
    for core in range(8):
        b = core // 2
        hg = core % 2
        xt = np.ascontiguousarray(x[b].T)  # [D, S]
        q_cols = slice(hg * 512, (hg + 1) * 512)
        k_cols = slice(D + hg * 512, D + (hg + 1) * 512)
        v_cols = slice(2 * D + hg * 512, 2 * D + (hg + 1) * 512)
        wq_s = np.ascontiguousarray(W_qkv[:, q_cols])
        wk_s = np.ascontiguousarray(W_qkv[:, k_cols])
        wv_s = np.ascontiguousarray(W_qkv[:, v_cols])
        if use_bias:
            xt = np.concatenate([xt, np.ones((1, S), np.float32)], axis=0)
            wq_s = np.concatenate([wq_s, b_qkv[None, hg * 512 : (hg + 1) * 512]], axis=0)
            wk_s = np.concatenate(
                [wk_s, b_qkv[None, D + hg * 512 : D + (hg + 1) * 512]], axis=0
            )
            wv_s = np.concatenate(
                [wv_s, b_qkv[None, 2 * D + hg * 512 : 2 * D + (hg + 1) * 512]], axis=0
            )
        wo_s = np.ascontiguousarray(W_out[hg * 512 : (hg + 1) * 512, :])
        in_maps.append(
            {
                "xT": np.ascontiguousarray(xt, dtype=np.float32),
                "wq": wq_s.astype(np.float32),
                "wk": wk_s.astype(np.float32),
                "wv": wv_s.astype(np.float32),
                "wo": wo_s.astype(np.float32),
                "mask": mask,
            }
        )
    return in_maps, use_bias


def gather_output(results, b_out):
    y = np.empty((B, S, D), dtype=np.float32)
    for b in range(B):
        y[b] = results[2 * b]["y"] + results[2 * b + 1]["y"]
    if b_out is not None and np.any(b_out):
        y += b_out[None, None, :].astype(np.float32)
    return y


def kernel(x, W_qkv, b_qkv, W_out, b_out):
    x = np.asarray(x, dtype=np.float32)
    W_qkv = np.asarray(W_qkv, dtype=np.float32)
    b_qkv = np.asarray(b_qkv, dtype=np.float32)
    W_out = np.asarray(W_out, dtype=np.float32)
    b_out = np.asarray(b_out, dtype=np.float32)
    in_maps, use_bias = make_in_maps(x, W_qkv, b_qkv, W_out)
    nc = _get_nc(use_bias)
    res = run_bass_kernel_spmd(nc, in_maps, core_ids=list(range(8)))
    return gather_output(res.results, b_out)
